# revision 7
# baseline (speedup 1.0000x reference)
"""Trainium2 Bass kernel for a dense transformer block (RMSNorm->MHA->res, RMSNorm->SwiGLU-FFN->res).

Sharding over 8 NeuronCores: fsdp=2 (batch) x tp=4 (attention heads / FFN hidden).
Core 4*b + t handles batch b with TP-rank t (heads 4t..4t+3, FFN hidden cols 2048t..2048(t+1)).

All on-device activations are feature-major ("transposed": [features, rows]) so every
matmul (out = lhsT.T @ rhs, contraction on the partition axis) chains without transposes:
  - q/k per head:   lhsT = W[e,d]    rhs = xnT[e,rows]   -> qT/kT [d, rows]
  - v (row-major):  lhsT = xnT[e,r]  rhs = Wv[e,(h d)]   -> v [rows, (h d)]
  - logitsT:        lhsT = kT[d,k]   rhs = qT[d,q]       -> [k, q]  (softmax denom via ones-matmul)
  - attn outT:      lhsT = v[k,d]    rhs = expT[k,q]     -> [d, q]
  - proj/ffn:       lhsT = W[in,out] rhs = actT[in,rows] -> [out, rows]
Matmul inputs use dtype float32r (full PE rate at N>=256); the residual stream stays float32.
scale1/scale2 of the RMSNorms are folded into the following weight matrices on the host.

Stage 1+2 stream in 512-row phases: phase qb computes rmsnorm+q/k/v for rows
[512qb, 512qb+512), then runs attention for q-block qb (causality only needs k/v
chunks <= qb, all computed). Head-shard attention output for q-block j goes straight
into AllToAll block j.

Collectives (within each 4-core TP group; all layouts rank-independent so the single
SPMD program works on every core — rank-dependent data arrives as per-core inputs):
  RS#1: out-proj partials (my heads, all rows) [4 x 2048(E) x 512] -> [2048, 512]
        (row-quarter t summed over the 4 head-shards onto rank t)
  AG:   rms2 rows [2048, 512] -> [4*2048, 512] (row-quarter blocks in rank order)
  RS#2: FFN partials [4*2048, 512] -> [2048, 512] (row-quarter t summed onto rank t)
"""

import numpy as np

EMBED = 2048
HEADS = 16
HEAD_DIM = 128
FF_HID = 8192
BATCH = 2
SEQ = 2048
EPS = 1e-6

N_CORES = 8
TP = 4
GROUPS = [[0, 1, 2, 3], [4, 5, 6, 7]]
H_LOC = HEADS // TP          # 4 heads per core
F_LOC = FF_HID // TP         # 2048 ffn-hidden per core
ROWS = SEQ                   # 2048 rows per batch
ROWS_T = ROWS // TP          # 512 rows per tp-rank
P = 128
NE = EMBED // P              # 16 embed chunks
NF = F_LOC // P              # 16 ffn chunks
NR = ROWS // P               # 16 row chunks
QB = 512                     # q-block / phase row count / matmul moving size
NQB = ROWS // QB             # 4 phases
RH = 1024                    # ffn row-half
INV_SQRT_D = float(1.0 / np.sqrt(HEAD_DIM))

_NC_CACHE = {}


def build_kernel():
    import concourse.mybir as mybir
    import concourse.tile as tile
    from concourse import bacc

    f32 = mybir.dt.float32

    nc = bacc.Bacc("TRN2", target_bir_lowering=False, debug=False, num_devices=N_CORES)

    io = {}
    io["xt"] = nc.dram_tensor("xt", [EMBED, ROWS], f32, kind="ExternalInput").ap()
    io["xtq"] = nc.dram_tensor("xtq", [EMBED, ROWS_T], f32, kind="ExternalInput").ap()
    io["wq"] = nc.dram_tensor("wq", [EMBED, H_LOC, HEAD_DIM], f32, kind="ExternalInput").ap()
    io["wk"] = nc.dram_tensor("wk", [EMBED, H_LOC, HEAD_DIM], f32, kind="ExternalInput").ap()
    io["wv"] = nc.dram_tensor("wv", [EMBED, H_LOC * HEAD_DIM], f32, kind="ExternalInput").ap()
    io["wout"] = nc.dram_tensor("wout", [H_LOC * HEAD_DIM, EMBED], f32, kind="ExternalInput").ap()
    io["wg"] = nc.dram_tensor("wg", [EMBED, F_LOC], f32, kind="ExternalInput").ap()
    io["wu"] = nc.dram_tensor("wu", [EMBED, F_LOC], f32, kind="ExternalInput").ap()
    io["wd"] = nc.dram_tensor("wd", [F_LOC, EMBED], f32, kind="ExternalInput").ap()
    io["masks"] = nc.dram_tensor("masks", [P, H_LOC, QB], f32, kind="ExternalInput").ap()
    io["ones"] = nc.dram_tensor("ones", [P, 1], f32, kind="ExternalInput").ap()
    io["out"] = nc.dram_tensor("out", [EMBED, ROWS_T], f32, kind="ExternalOutput").ap()

    with tile.TileContext(nc) as tc:
        _emit(tc, nc, io)
    nc.compile()
    return nc


def _emit(tc, nc, io):
    from contextlib import ExitStack

    import concourse.mybir as mybir

    f32 = mybir.dt.float32
    f32r = mybir.dt.float32r
    AF = mybir.ActivationFunctionType

    xt, xtq, wq, wk, wv = io["xt"], io["xtq"], io["wq"], io["wk"], io["wv"]
    ones_in = io["ones"]
    wout, wg, wu, wd, masks = io["wout"], io["wg"], io["wu"], io["wd"], io["masks"]
    out_ext = io["out"]

    def r3(ap2d, cols=None):
        """[(o p), q] dram view -> [p, o, q]; optionally slice columns first."""
        v = ap2d if cols is None else ap2d[:, cols]
        return v.rearrange("(o p) q -> p o q", p=P)

    ctx = ExitStack()
    with ctx:
        consts = ctx.enter_context(tc.tile_pool(name="consts", bufs=1))
        dram = ctx.enter_context(tc.tile_pool(name="dram", bufs=1, space="DRAM"))

        ones_sb = consts.tile([P, 1], f32r)
        nc.sync.dma_start(ones_sb[:], ones_in[:].bitcast(f32r))
        eps_sb = consts.tile([1, 1], f32)
        nc.vector.memset(eps_sb[:], EPS)

        rs1_in = dram.tile([TP * EMBED, ROWS_T], f32)
        rs1_out = dram.tile([EMBED, ROWS_T], f32)
        ag2_in = dram.tile([EMBED, ROWS_T], f32)
        ag2_out = dram.tile([TP * EMBED, ROWS_T], f32)
        rs_in = dram.tile([TP * EMBED, ROWS_T], f32)
        rs_out = dram.tile([EMBED, ROWS_T], f32)
        x2_scr = dram.tile([EMBED, ROWS_T], f32)

        # ========== Stage 1+2 (fused phases): rmsnorm1 + qkv + attention ==========
        with (
            tc.tile_pool(name="kv_store", bufs=1) as kv_pool,
            tc.tile_pool(name="s1", bufs=2) as s1,
            tc.tile_pool(name="s1ps", bufs=2, space="PSUM") as s1ps,
        ):
            k_store = kv_pool.tile([P, H_LOC, ROWS], f32r)
            v_store = kv_pool.tile([P, NR, H_LOC, HEAD_DIM], f32r)
            wv_sb = kv_pool.tile([P, NE, H_LOC * HEAD_DIM], f32r)
            nc.sync.dma_start(wv_sb[:], r3(wv).bitcast(f32r))
            mask_sb = kv_pool.tile([P, H_LOC, QB], f32r)
            nc.sync.dma_start(mask_sb[:], masks[:].bitcast(f32r))

            for qb in range(NQB):
                cols = slice(qb * QB, (qb + 1) * QB)
                xn = s1.tile([P, NE, QB], f32r, tag="xn", bufs=1)
                nc.sync.dma_start(xn[:], r3(xt, cols).bitcast(f32r))

                # rmsnorm stats
                ms_ps = s1ps.tile([1, QB], f32, tag="ms", bufs=1)
                for e in range(NE):
                    sq = s1.tile([P, QB], f32r, tag="sq", bufs=2)
                    sl = xn[:, e, :]
                    nc.vector.tensor_mul(sq[:], sl, sl)
                    nc.tensor.matmul(
                        ms_ps[:], ones_sb[:], sq[:],
                        start=(e == 0), stop=(e == NE - 1),
                    )
                rsq = s1.tile([1, QB], f32, tag="rsq", bufs=1)
                nc.scalar.activation(
                    rsq[:], ms_ps[:], AF.Sqrt, bias=eps_sb[:], scale=1.0 / EMBED
                )
                rsq_i = s1.tile([1, QB], f32, tag="rsqi", bufs=1)
                nc.vector.reciprocal(rsq_i[:], rsq[:])
                bc = s1.tile([P, QB], f32r, tag="bc", bufs=1)
                nc.gpsimd.partition_broadcast(bc[:].bitcast(f32), rsq_i[:])
                for e in range(NE):
                    sl = xn[:, e, :]
                    nc.vector.tensor_mul(sl, sl, bc[:])

                # q/k projections per head; q kept per-phase, k appended to k_store
                q_ph = s1.tile([P, H_LOC, QB], f32r, tag="q_ph", bufs=2)
                for h in range(H_LOC):
                    wq_sb = s1.tile([P, NE, HEAD_DIM], f32r, tag="wqk", bufs=2)
                    nc.sync.dma_start(wq_sb[:], r3(wq[:, h, :]).bitcast(f32r))
                    wk_sb = s1.tile([P, NE, HEAD_DIM], f32r, tag="wqk", bufs=2)
                    nc.sync.dma_start(wk_sb[:], r3(wk[:, h, :]).bitcast(f32r))
                    q_ps = s1ps.tile([P, QB], f32, tag="proj", bufs=2)
                    for e in range(NE):
                        nc.tensor.matmul(
                            q_ps[:], wq_sb[:, e, :], xn[:, e, :],
                            start=(e == 0), stop=(e == NE - 1),
                        )
                    nc.vector.tensor_copy(q_ph[:, h, :], q_ps[:])
                    k_ps = s1ps.tile([P, QB], f32, tag="proj", bufs=2)
                    for e in range(NE):
                        nc.tensor.matmul(
                            k_ps[:], wk_sb[:, e, :], xn[:, e, :],
                            start=(e == 0), stop=(e == NE - 1),
                        )
                    nc.scalar.activation(k_store[:, h, cols], k_ps[:], AF.Copy)
                # v projection: row-major, all 4 heads at once (N=512)
                for rc in range(QB // P):
                    rcg = qb * (QB // P) + rc
                    v_ps = s1ps.tile([P, H_LOC * HEAD_DIM], f32, tag="proj", bufs=2)
                    for e in range(NE):
                        nc.tensor.matmul(
                            v_ps[:], xn[:, e, rc * P : (rc + 1) * P], wv_sb[:, e, :],
                            start=(e == 0), stop=(e == NE - 1),
                        )
                    nc.vector.tensor_copy(
                        v_store[:, rcg].rearrange("p h d -> p (h d)"), v_ps[:]
                    )

                # ---- attention for q-block qb (k/v chunks 0..4qb+3 are ready) ----
                ao_ph = s1.tile([P, H_LOC, QB], f32r, tag="ao_ph", bufs=1)
                nk = (qb + 1) * (QB // P)
                for h in range(H_LOC):
                    pv_ps = s1ps.tile([P, QB], f32, tag="pv", bufs=2)
                    sum_ps = s1ps.tile([1, QB], f32, tag="ssum", bufs=1)
                    for kc in range(nk):
                        lg_ps = s1ps.tile([P, QB], f32, tag="lg", bufs=2)
                        nc.tensor.matmul(
                            lg_ps[:],
                            k_store[:, h, kc * P : (kc + 1) * P],
                            q_ph[:, h, :],
                            start=True, stop=True,
                        )
                        expt = s1.tile([P, QB], f32r, tag="expt", bufs=3)
                        nc.scalar.activation(expt[:], lg_ps[:], AF.Exp, scale=INV_SQRT_D)
                        j = kc - qb * (QB // P)
                        if j >= 0:
                            nc.vector.tensor_mul(expt[:], expt[:], mask_sb[:, j, :])
                        first, last = kc == 0, kc == nk - 1
                        nc.tensor.matmul(
                            pv_ps[:], v_store[:, kc, h, :], expt[:],
                            start=first, stop=last,
                        )
                        nc.tensor.matmul(
                            sum_ps[:], ones_sb[:], expt[:],
                            start=first, stop=last,
                        )
                    rec = s1.tile([1, QB], f32, tag="rec", bufs=1)
                    nc.vector.reciprocal(rec[:], sum_ps[:])
                    rbc = s1.tile([P, QB], f32r, tag="rbc", bufs=2)
                    nc.gpsimd.partition_broadcast(rbc[:].bitcast(f32), rec[:])
                    nc.vector.tensor_mul(ao_ph[:, h, :], pv_ps[:], rbc[:])

                # out-projection partial for this q-block from my 4 heads
                for e in range(NE):
                    wo_sb = s1.tile([P, H_LOC, P], f32r, tag="wo", bufs=2)
                    nc.sync.dma_start(
                        wo_sb[:], r3(wout, slice(e * P, (e + 1) * P)).bitcast(f32r)
                    )
                    pr_ps = s1ps.tile([P, QB], f32, tag="proj", bufs=2)
                    for c in range(H_LOC):
                        nc.tensor.matmul(
                            pr_ps[:], wo_sb[:, c, :], ao_ph[:, c, :],
                            start=(c == 0), stop=(c == H_LOC - 1),
                        )
                    pr_sb = s1.tile([P, QB], f32, tag="pr_sb", bufs=2)
                    nc.vector.tensor_copy(pr_sb[:], pr_ps[:])
                    nc.sync.dma_start(
                        r3(rs1_in[qb * EMBED + e * P : qb * EMBED + (e + 1) * P, :]),
                        pr_sb[:],
                    )

        nc.gpsimd.collective_compute(
            "ReduceScatter", mybir.AluOpType.add, replica_groups=GROUPS,
            ins=[rs1_in[:].opt()], outs=[rs1_out[:].opt()],
        )

        # ===== Stage 3+4: out-proj on my row quarter, residual, rmsnorm2 =====
        with (
            tc.tile_pool(name="s3", bufs=2) as s3,
            tc.tile_pool(name="s3ps", bufs=2, space="PSUM") as s3ps,
        ):
            ap_sb = s3.tile([P, NE, ROWS_T], f32, tag="ap", bufs=1)
            nc.sync.dma_start(ap_sb[:], r3(rs1_out))
            xq_sb = s3.tile([P, NE, ROWS_T], f32, tag="xq", bufs=1)
            nc.sync.dma_start(xq_sb[:], r3(xtq))
            x2 = s3.tile([P, NE, ROWS_T], f32, tag="x2", bufs=1)
            for e in range(NE):
                nc.vector.tensor_add(x2[:, e, :], ap_sb[:, e, :], xq_sb[:, e, :])
            nc.sync.dma_start(r3(x2_scr), x2[:])

            # rmsnorm2 on my 512 rows
            ms_ps = s3ps.tile([1, ROWS_T], f32, tag="ms2", bufs=1)
            for e in range(NE):
                sq = s3.tile([P, ROWS_T], f32r, tag="sq2", bufs=3)
                nc.vector.tensor_mul(sq[:], x2[:, e, :], x2[:, e, :])
                nc.tensor.matmul(
                    ms_ps[:], ones_sb[:], sq[:],
                    start=(e == 0), stop=(e == NE - 1),
                )
            rsq = s3.tile([1, ROWS_T], f32, tag="rsq2")
            nc.scalar.activation(
                rsq[:], ms_ps[:], AF.Sqrt, bias=eps_sb[:], scale=1.0 / EMBED
            )
            rsq_i = s3.tile([1, ROWS_T], f32, tag="rsqi2")
            nc.vector.reciprocal(rsq_i[:], rsq[:])
            bc = s3.tile([P, ROWS_T], f32r, tag="bc2")
            nc.gpsimd.partition_broadcast(bc[:].bitcast(f32), rsq_i[:])
            n2 = s3.tile([P, NE, ROWS_T], f32r, tag="n2", bufs=1)
            for e in range(NE):
                nc.vector.tensor_mul(n2[:, e, :], x2[:, e, :], bc[:])
            nc.sync.dma_start(r3(ag2_in).bitcast(f32r), n2[:])

        nc.gpsimd.collective_compute(
            "AllGather", mybir.AluOpType.bypass, replica_groups=GROUPS,
            ins=[ag2_in[:].opt()], outs=[ag2_out[:].opt()],
        )

        # ========== Stage 5: FFN (hid shard), partials to ReduceScatter ==========
        with (
            tc.tile_pool(name="s5", bufs=1) as s5,
            tc.tile_pool(name="s5t", bufs=2) as s5t,
            tc.tile_pool(name="s5ps", bufs=2, space="PSUM") as s5ps,
        ):
            for rhh in range(ROWS // RH):  # 1024-row halves
                n2_sb = s5.tile([P, NE, RH], f32r, tag="n2h")
                for g2 in range(RH // ROWS_T):
                    g = rhh * 2 + g2
                    nc.sync.dma_start(
                        n2_sb[:, :, g2 * ROWS_T : (g2 + 1) * ROWS_T],
                        r3(ag2_out[g * EMBED : (g + 1) * EMBED, :]).bitcast(f32r),
                    )
                act = s5.tile([P, NF, RH], f32r, tag="act")
                for f in range(NF):
                    wg_sb = s5t.tile([P, NE, P], f32r, tag="wgu", bufs=4)
                    nc.sync.dma_start(
                        wg_sb[:], r3(wg, slice(f * P, (f + 1) * P)).bitcast(f32r)
                    )
                    wu_sb = s5t.tile([P, NE, P], f32r, tag="wgu", bufs=4)
                    nc.sync.dma_start(
                        wu_sb[:], r3(wu, slice(f * P, (f + 1) * P)).bitcast(f32r)
                    )
                    for hb in range(2):
                        cols = slice(hb * QB, (hb + 1) * QB)
                        g_ps = s5ps.tile([P, QB], f32, tag="gate", bufs=2)
                        for e in range(NE):
                            nc.tensor.matmul(
                                g_ps[:], wg_sb[:, e, :], n2_sb[:, e, cols],
                                start=(e == 0), stop=(e == NE - 1),
                            )
                        u_ps = s5ps.tile([P, QB], f32, tag="up", bufs=2)
                        for e in range(NE):
                            nc.tensor.matmul(
                                u_ps[:], wu_sb[:, e, :], n2_sb[:, e, cols],
                                start=(e == 0), stop=(e == NE - 1),
                            )
                        gel = s5t.tile([P, QB], f32, tag="gel", bufs=3)
                        nc.scalar.activation(gel[:], g_ps[:], AF.Gelu_apprx_tanh)
                        nc.vector.tensor_mul(act[:, f, cols], gel[:], u_ps[:])
                for e in range(NE):
                    wd_sb = s5t.tile([P, NF, P], f32r, tag="wd", bufs=2)
                    nc.sync.dma_start(
                        wd_sb[:], r3(wd, slice(e * P, (e + 1) * P)).bitcast(f32r)
                    )
                    for hb in range(2):
                        g = rhh * 2 + hb
                        cols = slice(hb * QB, (hb + 1) * QB)
                        d_ps = s5ps.tile([P, QB], f32, tag="down", bufs=2)
                        for f in range(NF):
                            nc.tensor.matmul(
                                d_ps[:], wd_sb[:, f, :], act[:, f, cols],
                                start=(f == 0), stop=(f == NF - 1),
                            )
                        d_sb = s5t.tile([P, QB], f32, tag="dstage", bufs=3)
                        nc.vector.tensor_copy(d_sb[:], d_ps[:])
                        nc.sync.dma_start(
                            r3(rs_in[g * EMBED + e * P : g * EMBED + (e + 1) * P, :]),
                            d_sb[:],
                        )

        nc.gpsimd.collective_compute(
            "ReduceScatter", mybir.AluOpType.add, replica_groups=GROUPS,
            ins=[rs_in[:].opt()], outs=[rs_out[:].opt()],
        )

        # ================= Stage 6: final residual =================
        with tc.tile_pool(name="s6", bufs=1) as s6:
            fsum = s6.tile([P, NE, ROWS_T], f32, tag="fsum")
            nc.sync.dma_start(fsum[:], r3(rs_out))
            x2b = s6.tile([P, NE, ROWS_T], f32, tag="x2b")
            nc.sync.dma_start(x2b[:], r3(x2_scr))
            fin = s6.tile([P, NE, ROWS_T], f32, tag="fin")
            nc.vector.tensor_add(fin[:], fsum[:], x2b[:])
            nc.sync.dma_start(r3(out_ext), fin[:])


# ============================ host side ============================


def _prep_core_inputs(inputs):
    """Shard + transpose + fold rms scales into weights. Returns list of 8 in_maps."""
    x = np.asarray(inputs["x"], np.float32)          # [B, S, E]
    w_qkv = np.asarray(inputs["w_qkv"], np.float32)  # [E, H, 3D]
    w_out = np.asarray(inputs["w_out"], np.float32)  # [H, D, E]
    w_gate = np.asarray(inputs["w_gate"], np.float32)
    w_up = np.asarray(inputs["w_up"], np.float32)
    w_down = np.asarray(inputs["w_down"], np.float32)
    scale1 = np.asarray(inputs["scale1"], np.float32)
    scale2 = np.asarray(inputs["scale2"], np.float32)

    wqkv_s = w_qkv * scale1[:, None, None]
    wq_f = wqkv_s[:, :, 0:HEAD_DIM]
    wk_f = wqkv_s[:, :, HEAD_DIM : 2 * HEAD_DIM]
    wv_f = wqkv_s[:, :, 2 * HEAD_DIM : 3 * HEAD_DIM]
    wout_f = w_out.reshape(HEADS * HEAD_DIM, EMBED)
    wg_s = w_gate * scale2[:, None]
    wu_s = w_up * scale2[:, None]

    kp = np.arange(P)[:, None]
    qf = np.arange(QB)[None, :]
    masks = np.stack(
        [(qf >= kp + P * j).astype(np.float32) for j in range(H_LOC)], axis=1
    )  # [P, H_LOC, QB]

    in_maps = []
    for c in range(N_CORES):
        b, t = divmod(c, TP)
        hs = slice(H_LOC * t, H_LOC * (t + 1))
        fs = slice(F_LOC * t, F_LOC * (t + 1))
        xtb = np.ascontiguousarray(x[b].T)  # [E, S]
        in_maps.append(
            {
                "xt": xtb,
                "xtq": np.ascontiguousarray(xtb[:, ROWS_T * t : ROWS_T * (t + 1)]),
                "wq": np.ascontiguousarray(wq_f[:, hs, :]),
                "wk": np.ascontiguousarray(wk_f[:, hs, :]),
                "wv": np.ascontiguousarray(wv_f[:, hs, :].reshape(EMBED, H_LOC * HEAD_DIM)),
                "wout": np.ascontiguousarray(
                    wout_f[H_LOC * HEAD_DIM * t : H_LOC * HEAD_DIM * (t + 1), :]
                ),
                "wg": np.ascontiguousarray(wg_s[:, fs]),
                "wu": np.ascontiguousarray(wu_s[:, fs]),
                "wd": np.ascontiguousarray(w_down[fs, :]),
                "masks": np.ascontiguousarray(masks),
                "ones": np.ones((P, 1), np.float32),
            }
        )
    return in_maps


def _install_profile_hook():
    import sys
    import types

    try:
        import antenv.axon_hooks  # noqa: F401

        return
    except ImportError:
        pass
    try:
        from trn_agent_boot.trn_boot import _ntff_profile_via_ctypes

        _hook = _ntff_profile_via_ctypes("/opt/axon/libaxon_pjrt.so")
        _mod = types.ModuleType("antenv.axon_hooks")
        _mod.get_axon_ntff_profile_hook = lambda: _hook
        sys.modules["antenv.axon_hooks"] = _mod
    except Exception:
        pass


def _run(nc, in_maps, trace=False, trace_cores=None):
    _install_profile_hook()
    from concourse.bass_utils import run_bass_kernel_spmd

    return run_bass_kernel_spmd(
        nc,
        in_maps,
        core_ids=list(range(N_CORES)),
        trace=trace,
        trace_cores=trace_cores,
    )


def kernel(**inputs):
    if "nc" not in _NC_CACHE:
        _NC_CACHE["nc"] = build_kernel()
    nc = _NC_CACHE["nc"]
    in_maps = _prep_core_inputs(inputs)
    res = _run(nc, in_maps)
    out = np.empty((BATCH, SEQ, EMBED), np.float32)
    for c in range(N_CORES):
        b, t = divmod(c, TP)
        out[b, ROWS_T * t : ROWS_T * (t + 1), :] = res.results[c]["out"].T
    return out


if __name__ == "__main__":
    build_kernel()
    print("build ok")


# revision 12
# speedup vs baseline: 1.0288x; 1.0288x over previous
"""Trainium2 Bass kernel for a dense transformer block (RMSNorm->MHA->res, RMSNorm->SwiGLU-FFN->res).

Sharding over 8 NeuronCores: fsdp=2 (batch) x tp=4 (attention heads / FFN hidden).
Core 4*b + t handles batch b with TP-rank t (heads 4t..4t+3, FFN hidden cols 2048t..2048(t+1)).

All on-device activations are feature-major ("transposed": [features, rows]) so every
matmul (out = lhsT.T @ rhs, contraction on the partition axis) chains without transposes:
  - q/k per head:   lhsT = W[e,d]    rhs = xnT[e,rows]   -> qT/kT [d, rows]
  - v (row-major):  lhsT = xnT[e,r]  rhs = Wv[e,(h d)]   -> v [rows, (h d)]
  - logitsT:        lhsT = kT[d,k]   rhs = qT[d,q]       -> [k, q]  (softmax denom via ones-matmul)
  - attn outT:      lhsT = v[k,d]    rhs = expT[k,q]     -> [d, q]
  - proj/ffn:       lhsT = W[in,out] rhs = actT[in,rows] -> [out, rows]
Matmul inputs use dtype float32r (full PE rate at N>=256); the residual stream stays float32.
scale1/scale2 of the RMSNorms are folded into the following weight matrices on the host.

Stage 1+2 stream in 512-row phases: phase qb computes rmsnorm+q/k/v for rows
[512qb, 512qb+512), then runs attention for q-block qb (causality only needs k/v
chunks <= qb, all computed). Head-shard attention output for q-block j goes straight
into AllToAll block j.

Collectives (within each 4-core TP group; all layouts rank-independent so the single
SPMD program works on every core — rank-dependent data arrives as per-core inputs):
  RS#1: out-proj partials (my heads, all rows) [4 x 2048(E) x 512] -> [2048, 512]
        (row-quarter t summed over the 4 head-shards onto rank t)
  AG:   rms2 rows [2048, 512] -> [4*2048, 512] (row-quarter blocks in rank order)
  RS#2: FFN partials [4*2048, 512] -> [2048, 512] (row-quarter t summed onto rank t)
"""

import numpy as np

EMBED = 2048
HEADS = 16
HEAD_DIM = 128
FF_HID = 8192
BATCH = 2
SEQ = 2048
EPS = 1e-6

N_CORES = 8
TP = 4
GROUPS = [[0, 1, 2, 3], [4, 5, 6, 7]]
H_LOC = HEADS // TP          # 4 heads per core
F_LOC = FF_HID // TP         # 2048 ffn-hidden per core
ROWS = SEQ                   # 2048 rows per batch
ROWS_T = ROWS // TP          # 512 rows per tp-rank
P = 128
NE = EMBED // P              # 16 embed chunks
NF = F_LOC // P              # 16 ffn chunks
NR = ROWS // P               # 16 row chunks
QB = 512                     # q-block / phase row count / matmul moving size
NQB = ROWS // QB             # 4 phases
RH = 1024                    # ffn row-half
INV_SQRT_D = float(1.0 / np.sqrt(HEAD_DIM))

_NC_CACHE = {}


def build_kernel():
    import concourse.mybir as mybir
    import concourse.tile as tile
    from concourse import bacc

    f32 = mybir.dt.float32

    nc = bacc.Bacc("TRN2", target_bir_lowering=False, debug=False, num_devices=N_CORES)

    io = {}
    io["xt"] = nc.dram_tensor("xt", [EMBED, ROWS], f32, kind="ExternalInput").ap()
    io["xtq"] = nc.dram_tensor("xtq", [EMBED, ROWS_T], f32, kind="ExternalInput").ap()
    io["wq"] = nc.dram_tensor("wq", [EMBED, H_LOC, HEAD_DIM], f32, kind="ExternalInput").ap()
    io["wk"] = nc.dram_tensor("wk", [EMBED, H_LOC, HEAD_DIM], f32, kind="ExternalInput").ap()
    io["wv"] = nc.dram_tensor("wv", [EMBED, H_LOC * HEAD_DIM], f32, kind="ExternalInput").ap()
    io["wout"] = nc.dram_tensor("wout", [H_LOC * HEAD_DIM, EMBED], f32, kind="ExternalInput").ap()
    io["wg"] = nc.dram_tensor("wg", [EMBED, F_LOC], f32, kind="ExternalInput").ap()
    io["wu"] = nc.dram_tensor("wu", [EMBED, F_LOC], f32, kind="ExternalInput").ap()
    io["wd"] = nc.dram_tensor("wd", [F_LOC, EMBED], f32, kind="ExternalInput").ap()
    io["masks"] = nc.dram_tensor("masks", [P, QB + 3 * P], f32, kind="ExternalInput").ap()
    io["ones"] = nc.dram_tensor("ones", [P, 1], f32, kind="ExternalInput").ap()
    io["out"] = nc.dram_tensor("out", [EMBED, ROWS_T], f32, kind="ExternalOutput").ap()

    with tile.TileContext(nc) as tc:
        _emit(tc, nc, io)
    nc.compile()
    return nc


def _emit(tc, nc, io):
    from contextlib import ExitStack

    import concourse.mybir as mybir

    f32 = mybir.dt.float32
    f32r = mybir.dt.float32r
    AF = mybir.ActivationFunctionType

    xt, xtq, wq, wk, wv = io["xt"], io["xtq"], io["wq"], io["wk"], io["wv"]
    ones_in = io["ones"]
    wout, wg, wu, wd, masks = io["wout"], io["wg"], io["wu"], io["wd"], io["masks"]
    out_ext = io["out"]

    def r3(ap2d, cols=None):
        """[(o p), q] dram view -> [p, o, q]; optionally slice columns first."""
        v = ap2d if cols is None else ap2d[:, cols]
        return v.rearrange("(o p) q -> p o q", p=P)

    ctx = ExitStack()
    with ctx:
        consts = ctx.enter_context(tc.tile_pool(name="consts", bufs=1))
        dram = ctx.enter_context(tc.tile_pool(name="dram", bufs=1, space="DRAM"))

        ones_sb = consts.tile([P, 1], f32r)
        nc.sync.dma_start(ones_sb[:], ones_in[:].bitcast(f32r))
        eps_sb = consts.tile([1, 1], f32)
        nc.vector.memset(eps_sb[:], EPS)

        rs1_in = dram.tile([TP * EMBED, ROWS_T], f32)
        rs1_out = dram.tile([EMBED, ROWS_T], f32)
        ag2_in = dram.tile([EMBED, ROWS_T], f32)
        ag2_out = dram.tile([TP * EMBED, ROWS_T], f32)
        rs_in = dram.tile([TP * EMBED, ROWS_T], f32)
        rs_out = dram.tile([EMBED, ROWS_T], f32)
        x2_scr = dram.tile([EMBED, ROWS_T], f32)

        # ========== Stage 1+2 (fused phases): rmsnorm1 + qkv + attention ==========
        # Phase qb: rms+qkv for rows [512qb,512qb+512) then attention for q-block qb.
        # The next phase's square/mean chain is interleaved with this phase's
        # out-projection so the PE never starves on DVE latency; logits matmuls are
        # emitted one k-chunk ahead of the exp/mask chain.
        # rs1_in layout: row = (e//4)*2048 + qb*512 + (e%4)*128 + p, so each e-range
        # (er) forms a contiguous [2048, 512] block that ReduceScatters on its own as
        # soon as phase 3 finishes that e-range.
        with (
            tc.tile_pool(name="kv_store", bufs=1) as kv_pool,
            tc.tile_pool(name="s1", bufs=2) as s1,
            tc.tile_pool(name="s1ps", bufs=2, space="PSUM") as s1ps,
        ):
            k_store = kv_pool.tile([P, H_LOC, ROWS], f32r)
            v_store = kv_pool.tile([P, NR, H_LOC, HEAD_DIM], f32r)
            mask_sb = kv_pool.tile([P, QB + 3 * P], f32r)
            nc.sync.dma_start(mask_sb[:], masks[:].bitcast(f32r))

            xns = {}

            def emit_xn_dma(qb):
                xn = s1.tile([P, NE, QB], f32r, tag="xn", bufs=1, name=f"xn{qb}")
                nc.sync.dma_start(xn[:], r3(xt, slice(qb * QB, (qb + 1) * QB)).bitcast(f32r))
                xns[qb] = xn

            def emit_sq_ms_step(qb, e):
                """square + mean-accumulate for chunk e of phase qb."""
                if e == 0:
                    ms = s1ps.tile([1, QB], f32, tag="acc1", bufs=2, name=f"ms{qb}")
                    xns[(qb, "ms")] = ms
                ms = xns[(qb, "ms")]
                sq = s1.tile([P, QB], f32r, tag="sq", bufs=2)
                sl = xns[qb][:, e, :]
                nc.vector.tensor_mul(sq[:], sl, sl)
                nc.tensor.matmul(ms[:], ones_sb[:], sq[:],
                                 start=(e == 0), stop=(e == NE - 1))

            def emit_norm_tail(qb):
                ms = xns.pop((qb, "ms"))
                rsq = s1.tile([1, QB], f32, tag="rsq", bufs=1)
                nc.scalar.activation(rsq[:], ms[:], AF.Sqrt, bias=eps_sb[:], scale=1.0 / EMBED)
                rsq_i = s1.tile([1, QB], f32, tag="rsqi", bufs=1)
                nc.vector.reciprocal(rsq_i[:], rsq[:])
                bc = s1.tile([P, QB], f32r, tag="bc", bufs=1)
                nc.gpsimd.partition_broadcast(bc[:].bitcast(f32), rsq_i[:])
                xn = xns[qb]
                for e in range(NE):
                    sl = xn[:, e, :]
                    nc.vector.tensor_mul(sl, sl, bc[:])

            def emit_qkv(qb):
                xn = xns[qb]
                cols = slice(qb * QB, (qb + 1) * QB)
                q_ph = s1.tile([P, H_LOC, QB], f32r, tag="q_ph", bufs=2, name=f"q{qb}")
                for h in range(H_LOC):
                    wq_sb = s1.tile([P, NE, HEAD_DIM], f32r, tag="wqk", bufs=4)
                    nc.sync.dma_start(wq_sb[:], r3(wq[:, h, :]).bitcast(f32r))
                    wk_sb = s1.tile([P, NE, HEAD_DIM], f32r, tag="wqk", bufs=4)
                    nc.sync.dma_start(wk_sb[:], r3(wk[:, h, :]).bitcast(f32r))
                    q_ps = s1ps.tile([P, QB], f32, tag="proj", bufs=2)
                    for e in range(NE):
                        nc.tensor.matmul(q_ps[:], wq_sb[:, e, :], xn[:, e, :],
                                         start=(e == 0), stop=(e == NE - 1))
                    nc.vector.tensor_copy(q_ph[:, h, :], q_ps[:])
                    k_ps = s1ps.tile([P, QB], f32, tag="proj", bufs=2)
                    for e in range(NE):
                        nc.tensor.matmul(k_ps[:], wk_sb[:, e, :], xn[:, e, :],
                                         start=(e == 0), stop=(e == NE - 1))
                    nc.scalar.activation(k_store[:, h, cols], k_ps[:], AF.Copy)
                # v: e-outer with wv streamed; 4 row-chunk accumulators borrow
                # the lg/pv PSUM slots (idle between attention blocks)
                v_ps = [
                    s1ps.tile([P, H_LOC * HEAD_DIM], f32, tag=t, bufs=2,
                              name=f"v_ps{i}")
                    for i, t in enumerate(("lg", "lg", "pv", "pv"))
                ]
                for e in range(NE):
                    wv_e = s1.tile([P, H_LOC * HEAD_DIM], f32r, tag="wv_e", bufs=3)
                    nc.sync.dma_start(
                        wv_e[:], r3(wv)[:, e, :].bitcast(f32r))
                    for rc in range(QB // P):
                        nc.tensor.matmul(v_ps[rc][:], xn[:, e, rc * P : (rc + 1) * P],
                                         wv_e[:],
                                         start=(e == 0), stop=(e == NE - 1))
                for rc in range(QB // P):
                    rcg = qb * (QB // P) + rc
                    nc.vector.tensor_copy(
                        v_store[:, rcg].rearrange("p h d -> p (h d)"), v_ps[rc][:])
                return q_ph

            def emit_attention(qb, q_ph):
                ao_ph = s1.tile([P, H_LOC, QB], f32r, tag="ao_ph", bufs=1, name=f"ao{qb}")
                nk = (qb + 1) * (QB // P)
                for h in range(H_LOC):
                    pv_ps = s1ps.tile([P, QB], f32, tag="pv", bufs=2)
                    sum_ps = s1ps.tile([1, QB], f32, tag="acc1", bufs=2)
                    lg_tiles = {}

                    def emit_lg(kc):
                        lg = s1ps.tile([P, QB], f32, tag="lg", bufs=2)
                        nc.tensor.matmul(
                            lg[:], k_store[:, h, kc * P : (kc + 1) * P],
                            q_ph[:, h, :], start=True, stop=True)
                        lg_tiles[kc] = lg

                    emit_lg(0)
                    for kc in range(nk):
                        if kc + 1 < nk:
                            emit_lg(kc + 1)
                        lg = lg_tiles.pop(kc)
                        expt = s1.tile([P, QB], f32r, tag="expt", bufs=3)
                        nc.scalar.activation(expt[:], lg[:], AF.Exp, scale=INV_SQRT_D)
                        j = kc - qb * (QB // P)
                        if j >= 0:
                            off = (3 - j) * P
                            nc.vector.tensor_mul(expt[:], expt[:],
                                                 mask_sb[:, off : off + QB])
                        first, last = kc == 0, kc == nk - 1
                        nc.tensor.matmul(pv_ps[:], v_store[:, kc, h, :], expt[:],
                                         start=first, stop=last)
                        nc.tensor.matmul(sum_ps[:], ones_sb[:], expt[:],
                                         start=first, stop=last)
                    rec = s1.tile([1, QB], f32, tag="rec", bufs=2)
                    nc.vector.reciprocal(rec[:], sum_ps[:])
                    rbc = s1.tile([P, QB], f32r, tag="rbc", bufs=2)
                    nc.gpsimd.partition_broadcast(rbc[:].bitcast(f32), rec[:])
                    nc.vector.tensor_mul(ao_ph[:, h, :], pv_ps[:], rbc[:])
                return ao_ph

            def emit_outproj_step(qb, e, ao_ph):
                """one e-chunk of the out-projection partials of phase qb."""
                wo_sb = s1.tile([P, H_LOC, P], f32r, tag="wo", bufs=3)
                nc.sync.dma_start(
                    wo_sb[:], r3(wout, slice(e * P, (e + 1) * P)).bitcast(f32r))
                pr_ps = s1ps.tile([P, QB], f32, tag="proj", bufs=2)
                for c in range(H_LOC):
                    nc.tensor.matmul(pr_ps[:], wo_sb[:, c, :], ao_ph[:, c, :],
                                     start=(c == 0), stop=(c == H_LOC - 1))
                pr_sb = s1.tile([P, QB], f32, tag="pr_sb", bufs=2)
                nc.vector.tensor_copy(pr_sb[:], pr_ps[:])
                er, em = divmod(e, 4)
                row = er * TP * QB + qb * QB + em * P
                nc.sync.dma_start(r3(rs1_in[row : row + P, :]), pr_sb[:])

            def emit_rs1(er):
                nc.gpsimd.collective_compute(
                    "ReduceScatter", mybir.AluOpType.add, replica_groups=GROUPS,
                    ins=[rs1_in[er * TP * QB : (er + 1) * TP * QB, :].opt()],
                    outs=[rs1_out[er * QB : (er + 1) * QB, :].opt()],
                )

            # ---- phase schedule ----
            emit_xn_dma(0)
            for e in range(NE):
                emit_sq_ms_step(0, e)
            emit_norm_tail(0)
            aos = {}
            for qb in range(NQB):
                q_ph = emit_qkv(qb)
                aos[qb] = emit_attention(qb, q_ph)
                if qb + 1 < NQB:
                    # interleave next phase's rms chain with this phase's out-proj
                    emit_xn_dma(qb + 1)
                    for e in range(NE):
                        emit_sq_ms_step(qb + 1, e)
                        emit_outproj_step(qb, e, aos[qb])
                    emit_norm_tail(qb + 1)
                else:
                    for e in range(NE):
                        emit_outproj_step(qb, e, aos[qb])
                        if e % 4 == 3:
                            emit_rs1(e // 4)

        # ===== Stage 3+4: out-proj on my row quarter, residual, rmsnorm2 =====
        with (
            tc.tile_pool(name="s3", bufs=2) as s3,
            tc.tile_pool(name="s3ps", bufs=2, space="PSUM") as s3ps,
        ):
            ap_sb = s3.tile([P, NE, ROWS_T], f32, tag="ap", bufs=1)
            xq_sb = s3.tile([P, NE, ROWS_T], f32, tag="xq", bufs=1)
            nc.sync.dma_start(xq_sb[:], r3(xtq))
            x2 = s3.tile([P, NE, ROWS_T], f32, tag="x2", bufs=1)
            ms_ps = s3ps.tile([1, ROWS_T], f32, tag="ms2", bufs=1)
            for er in range(4):
                nc.sync.dma_start(
                    ap_sb[:, er * 4 : (er + 1) * 4, :],
                    r3(rs1_out[er * ROWS_T : (er + 1) * ROWS_T, :]),
                )
                for em in range(4):
                    e = er * 4 + em
                    nc.vector.tensor_add(x2[:, e, :], ap_sb[:, e, :], xq_sb[:, e, :])
                    sq = s3.tile([P, ROWS_T], f32r, tag="sq2", bufs=3)
                    nc.vector.tensor_mul(sq[:], x2[:, e, :], x2[:, e, :])
                    nc.tensor.matmul(
                        ms_ps[:], ones_sb[:], sq[:],
                        start=(e == 0), stop=(e == NE - 1),
                    )
            nc.sync.dma_start(r3(x2_scr), x2[:])
            rsq = s3.tile([1, ROWS_T], f32, tag="rsq2")
            nc.scalar.activation(
                rsq[:], ms_ps[:], AF.Sqrt, bias=eps_sb[:], scale=1.0 / EMBED
            )
            rsq_i = s3.tile([1, ROWS_T], f32, tag="rsqi2")
            nc.vector.reciprocal(rsq_i[:], rsq[:])
            bc = s3.tile([P, ROWS_T], f32r, tag="bc2")
            nc.gpsimd.partition_broadcast(bc[:].bitcast(f32), rsq_i[:])
            n2 = s3.tile([P, NE, ROWS_T], f32r, tag="n2", bufs=1)
            for e in range(NE):
                nc.vector.tensor_mul(n2[:, e, :], x2[:, e, :], bc[:])
            nc.sync.dma_start(r3(ag2_in).bitcast(f32r), n2[:])

        nc.gpsimd.collective_compute(
            "AllGather", mybir.AluOpType.bypass, replica_groups=GROUPS,
            ins=[ag2_in[:].opt()], outs=[ag2_out[:].opt()],
        )

        # ========== Stage 5: FFN (hid shard), partials to ReduceScatter ==========
        with (
            tc.tile_pool(name="s5", bufs=1) as s5,
            tc.tile_pool(name="s5t", bufs=2) as s5t,
            tc.tile_pool(name="s5ps", bufs=2, space="PSUM") as s5ps,
        ):
            for rhh in range(ROWS // RH):  # 1024-row halves
                n2_sb = s5.tile([P, NE, RH], f32r, tag="n2h")
                for g2 in range(RH // ROWS_T):
                    g = rhh * 2 + g2
                    nc.sync.dma_start(
                        n2_sb[:, :, g2 * ROWS_T : (g2 + 1) * ROWS_T],
                        r3(ag2_out[g * EMBED : (g + 1) * EMBED, :]).bitcast(f32r),
                    )
                act = s5.tile([P, NF, RH], f32r, tag="act")
                for f in range(NF):
                    wg_sb = s5t.tile([P, NE, P], f32r, tag="wgu", bufs=4)
                    nc.sync.dma_start(
                        wg_sb[:], r3(wg, slice(f * P, (f + 1) * P)).bitcast(f32r)
                    )
                    wu_sb = s5t.tile([P, NE, P], f32r, tag="wgu", bufs=4)
                    nc.sync.dma_start(
                        wu_sb[:], r3(wu, slice(f * P, (f + 1) * P)).bitcast(f32r)
                    )
                    for hb in range(2):
                        cols = slice(hb * QB, (hb + 1) * QB)
                        g_ps = s5ps.tile([P, QB], f32, tag="gate", bufs=2)
                        for e in range(NE):
                            nc.tensor.matmul(
                                g_ps[:], wg_sb[:, e, :], n2_sb[:, e, cols],
                                start=(e == 0), stop=(e == NE - 1),
                            )
                        u_ps = s5ps.tile([P, QB], f32, tag="up", bufs=2)
                        for e in range(NE):
                            nc.tensor.matmul(
                                u_ps[:], wu_sb[:, e, :], n2_sb[:, e, cols],
                                start=(e == 0), stop=(e == NE - 1),
                            )
                        gel = s5t.tile([P, QB], f32, tag="gel", bufs=3)
                        nc.scalar.activation(gel[:], g_ps[:], AF.Gelu_apprx_tanh)
                        nc.vector.tensor_mul(act[:, f, cols], gel[:], u_ps[:])
                for e in range(NE):
                    wd_sb = s5t.tile([P, NF, P], f32r, tag="wd", bufs=2)
                    nc.sync.dma_start(
                        wd_sb[:], r3(wd, slice(e * P, (e + 1) * P)).bitcast(f32r)
                    )
                    er, em = divmod(e, 4)
                    for hb in range(2):
                        g = rhh * 2 + hb
                        cols = slice(hb * QB, (hb + 1) * QB)
                        d_ps = s5ps.tile([P, QB], f32, tag="down", bufs=2)
                        for f in range(NF):
                            nc.tensor.matmul(
                                d_ps[:], wd_sb[:, f, :], act[:, f, cols],
                                start=(f == 0), stop=(f == NF - 1),
                            )
                        d_sb = s5t.tile([P, QB], f32, tag="dstage", bufs=3)
                        nc.vector.tensor_copy(d_sb[:], d_ps[:])
                        row = er * TP * QB + g * QB + em * P
                        nc.sync.dma_start(r3(rs_in[row : row + P, :]), d_sb[:])
                    if rhh == 1 and em == 3:
                        nc.gpsimd.collective_compute(
                            "ReduceScatter", mybir.AluOpType.add,
                            replica_groups=GROUPS,
                            ins=[rs_in[er * TP * QB : (er + 1) * TP * QB, :].opt()],
                            outs=[rs_out[er * QB : (er + 1) * QB, :].opt()],
                        )

        # ================= Stage 6: final residual (per e-range) =================
        with tc.tile_pool(name="s6", bufs=2) as s6:
            for er in range(4):
                rows = slice(er * ROWS_T, (er + 1) * ROWS_T)
                fsum = s6.tile([P, 4, ROWS_T], f32, tag="fsum", bufs=2)
                nc.sync.dma_start(fsum[:], r3(rs_out[rows, :]))
                x2b = s6.tile([P, 4, ROWS_T], f32, tag="x2b", bufs=2)
                nc.sync.dma_start(x2b[:], r3(x2_scr[rows, :]))
                fin = s6.tile([P, 4, ROWS_T], f32, tag="fin", bufs=2)
                nc.vector.tensor_add(fin[:], fsum[:], x2b[:])
                nc.sync.dma_start(r3(out_ext[rows, :]), fin[:])


# ============================ host side ============================


def _prep_core_inputs(inputs):
    """Shard + transpose + fold rms scales into weights. Returns list of 8 in_maps."""
    x = np.asarray(inputs["x"], np.float32)          # [B, S, E]
    w_qkv = np.asarray(inputs["w_qkv"], np.float32)  # [E, H, 3D]
    w_out = np.asarray(inputs["w_out"], np.float32)  # [H, D, E]
    w_gate = np.asarray(inputs["w_gate"], np.float32)
    w_up = np.asarray(inputs["w_up"], np.float32)
    w_down = np.asarray(inputs["w_down"], np.float32)
    scale1 = np.asarray(inputs["scale1"], np.float32)
    scale2 = np.asarray(inputs["scale2"], np.float32)

    wqkv_s = w_qkv * scale1[:, None, None]
    wq_f = wqkv_s[:, :, 0:HEAD_DIM]
    wk_f = wqkv_s[:, :, HEAD_DIM : 2 * HEAD_DIM]
    wv_f = wqkv_s[:, :, 2 * HEAD_DIM : 3 * HEAD_DIM]
    wout_f = w_out.reshape(HEADS * HEAD_DIM, EMBED)
    wg_s = w_gate * scale2[:, None]
    wu_s = w_up * scale2[:, None]

    kp = np.arange(P)[:, None]
    m = np.arange(QB + 3 * P)[None, :]
    masks = (m >= kp + 3 * P).astype(np.float32)  # mask_j = masks[:, (3-j)*128 : (3-j)*128+512]

    in_maps = []
    for c in range(N_CORES):
        b, t = divmod(c, TP)
        hs = slice(H_LOC * t, H_LOC * (t + 1))
        fs = slice(F_LOC * t, F_LOC * (t + 1))
        xtb = np.ascontiguousarray(x[b].T)  # [E, S]
        in_maps.append(
            {
                "xt": xtb,
                "xtq": np.ascontiguousarray(xtb[:, ROWS_T * t : ROWS_T * (t + 1)]),
                "wq": np.ascontiguousarray(wq_f[:, hs, :]),
                "wk": np.ascontiguousarray(wk_f[:, hs, :]),
                "wv": np.ascontiguousarray(wv_f[:, hs, :].reshape(EMBED, H_LOC * HEAD_DIM)),
                "wout": np.ascontiguousarray(
                    wout_f[H_LOC * HEAD_DIM * t : H_LOC * HEAD_DIM * (t + 1), :]
                ),
                "wg": np.ascontiguousarray(wg_s[:, fs]),
                "wu": np.ascontiguousarray(wu_s[:, fs]),
                "wd": np.ascontiguousarray(w_down[fs, :]),
                "masks": np.ascontiguousarray(masks),
                "ones": np.ones((P, 1), np.float32),
            }
        )
    return in_maps


def _install_profile_hook():
    import sys
    import types

    try:
        import antenv.axon_hooks  # noqa: F401

        return
    except ImportError:
        pass
    try:
        from trn_agent_boot.trn_boot import _ntff_profile_via_ctypes

        _hook = _ntff_profile_via_ctypes("/opt/axon/libaxon_pjrt.so")
        _mod = types.ModuleType("antenv.axon_hooks")
        _mod.get_axon_ntff_profile_hook = lambda: _hook
        sys.modules["antenv.axon_hooks"] = _mod
    except Exception:
        pass


def _run(nc, in_maps, trace=False, trace_cores=None):
    _install_profile_hook()
    from concourse.bass_utils import run_bass_kernel_spmd

    return run_bass_kernel_spmd(
        nc,
        in_maps,
        core_ids=list(range(N_CORES)),
        trace=trace,
        trace_cores=trace_cores,
    )


def kernel(**inputs):
    if "nc" not in _NC_CACHE:
        _NC_CACHE["nc"] = build_kernel()
    nc = _NC_CACHE["nc"]
    in_maps = _prep_core_inputs(inputs)
    res = _run(nc, in_maps)
    out = np.empty((BATCH, SEQ, EMBED), np.float32)
    for c in range(N_CORES):
        b, t = divmod(c, TP)
        out[b, ROWS_T * t : ROWS_T * (t + 1), :] = res.results[c]["out"].T
    return out


if __name__ == "__main__":
    build_kernel()
    print("build ok")


# revision 16
# speedup vs baseline: 1.0569x; 1.0272x over previous
"""Trainium2 Bass kernel for a dense transformer block (RMSNorm->MHA->res, RMSNorm->SwiGLU-FFN->res).

Sharding over 8 NeuronCores: fsdp=2 (batch) x tp=4 (attention heads / FFN hidden).
Core 4*b + t handles batch b with TP-rank t (heads 4t..4t+3, FFN hidden cols 2048t..2048(t+1)).

All on-device activations are feature-major ("transposed": [features, rows]) so every
matmul (out = lhsT.T @ rhs, contraction on the partition axis) chains without transposes:
  - q/k per head:   lhsT = W[e,d]    rhs = xnT[e,rows]   -> qT/kT [d, rows]
  - v (row-major):  lhsT = xnT[e,r]  rhs = Wv[e,(h d)]   -> v [rows, (h d)]
  - logitsT:        lhsT = kT[d,k]   rhs = qT[d,q]       -> [k, q]  (softmax denom via ones-matmul)
  - attn outT:      lhsT = v[k,d]    rhs = expT[k,q]     -> [d, q]
  - proj/ffn:       lhsT = W[in,out] rhs = actT[in,rows] -> [out, rows]
Matmul inputs use dtype float32r (full PE rate at N>=256); the residual stream stays float32.
scale1/scale2 of the RMSNorms are folded into the following weight matrices on the host.

Stage 1+2 stream in 512-row phases: phase qb computes rmsnorm+q/k/v for rows
[512qb, 512qb+512), then runs attention for q-block qb (causality only needs k/v
chunks <= qb, all computed). Head-shard attention output for q-block j goes straight
into AllToAll block j.

Collectives (within each 4-core TP group; all layouts rank-independent so the single
SPMD program works on every core — rank-dependent data arrives as per-core inputs).
Both ReduceScatters scatter along the E (embed) axis, so each rank owns an E-slice of
the summed result; this lets RS#1(qb) fire right after phase qb and hide under later
phases, and RS#2(g) fire per row-group under later FFN work:
  RS#1(qb): phase-qb out-proj partials [4 x 512(E) x 512] -> [512(my E), 512 rows]
  AR_ms(qb): [1, 512] rms2 mean-square partial sums (tiny)
  AG#2 (x2 halves): rms2'd rows [512(my E), 1024] -> [2048(all E), 1024]
  RS#2(g): FFN partials for row-group g [4 x 512(E) x 512] -> [512(my E), 512]
Final output per core: [my 512 E-features, all 2048 rows] of its batch.
"""

import numpy as np

EMBED = 2048
HEADS = 16
HEAD_DIM = 128
FF_HID = 8192
BATCH = 2
SEQ = 2048
EPS = 1e-6

N_CORES = 8
TP = 4
GROUPS = [[0, 1, 2, 3], [4, 5, 6, 7]]
H_LOC = HEADS // TP          # 4 heads per core
F_LOC = FF_HID // TP         # 2048 ffn-hidden per core
ROWS = SEQ                   # 2048 rows per batch
ROWS_T = ROWS // TP          # 512 rows per tp-rank
P = 128
NE = EMBED // P              # 16 embed chunks
NF = F_LOC // P              # 16 ffn chunks
NR = ROWS // P               # 16 row chunks
QB = 512                     # q-block / phase row count / matmul moving size
NQB = ROWS // QB             # 4 phases
RH = 1024                    # ffn row-half
INV_SQRT_D = float(1.0 / np.sqrt(HEAD_DIM))

_NC_CACHE = {}


def build_kernel():
    import concourse.mybir as mybir
    import concourse.tile as tile
    from concourse import bacc

    f32 = mybir.dt.float32

    nc = bacc.Bacc("TRN2", target_bir_lowering=False, debug=False, num_devices=N_CORES)

    io = {}
    io["xt"] = nc.dram_tensor("xt", [EMBED, ROWS], f32, kind="ExternalInput").ap()
    io["xte"] = nc.dram_tensor("xte", [ROWS_T, ROWS], f32, kind="ExternalInput").ap()
    io["wq"] = nc.dram_tensor("wq", [EMBED, H_LOC, HEAD_DIM], f32, kind="ExternalInput").ap()
    io["wk"] = nc.dram_tensor("wk", [EMBED, H_LOC, HEAD_DIM], f32, kind="ExternalInput").ap()
    io["wv"] = nc.dram_tensor("wv", [EMBED, H_LOC * HEAD_DIM], f32, kind="ExternalInput").ap()
    io["wout"] = nc.dram_tensor("wout", [H_LOC * HEAD_DIM, EMBED], f32, kind="ExternalInput").ap()
    io["wg"] = nc.dram_tensor("wg", [EMBED, F_LOC], f32, kind="ExternalInput").ap()
    io["wu"] = nc.dram_tensor("wu", [EMBED, F_LOC], f32, kind="ExternalInput").ap()
    io["wd"] = nc.dram_tensor("wd", [F_LOC, EMBED], f32, kind="ExternalInput").ap()
    io["masks"] = nc.dram_tensor("masks", [P, QB + 3 * P], f32, kind="ExternalInput").ap()
    io["ones"] = nc.dram_tensor("ones", [P, 1], f32, kind="ExternalInput").ap()
    io["out"] = nc.dram_tensor("out", [ROWS_T, ROWS], f32, kind="ExternalOutput").ap()

    with tile.TileContext(nc) as tc:
        _emit(tc, nc, io)
    nc.compile()
    return nc


def _emit(tc, nc, io):
    from contextlib import ExitStack

    import concourse.mybir as mybir

    f32 = mybir.dt.float32
    f32r = mybir.dt.float32r
    AF = mybir.ActivationFunctionType

    xt, xte, wq, wk, wv = io["xt"], io["xte"], io["wq"], io["wk"], io["wv"]
    ones_in = io["ones"]
    wout, wg, wu, wd, masks = io["wout"], io["wg"], io["wu"], io["wd"], io["masks"]
    out_ext = io["out"]

    def r3(ap2d, cols=None):
        """[(o p), q] dram view -> [p, o, q]; optionally slice columns first."""
        v = ap2d if cols is None else ap2d[:, cols]
        return v.rearrange("(o p) q -> p o q", p=P)

    ctx = ExitStack()
    with ctx:
        consts = ctx.enter_context(tc.tile_pool(name="consts", bufs=1))
        dram = ctx.enter_context(tc.tile_pool(name="dram", bufs=1, space="DRAM"))

        ones_sb = consts.tile([P, 1], f32r)
        nc.sync.dma_start(ones_sb[:], ones_in[:].bitcast(f32r))
        eps_sb = consts.tile([1, 1], f32)
        nc.vector.memset(eps_sb[:], EPS)

        rs1_in = dram.tile([NQB, EMBED, ROWS_T], f32)
        rs1_out = dram.tile([NQB, ROWS_T, ROWS_T], f32)
        ar_in = dram.tile([NQB, 1, ROWS_T], f32)
        ar_out = dram.tile([NQB, 1, ROWS_T], f32)
        ag2a_in = dram.tile([ROWS_T, RH], f32)
        ag2a_out = dram.tile([EMBED, RH], f32)
        ag2b_in = dram.tile([ROWS_T, RH], f32)
        ag2b_out = dram.tile([EMBED, RH], f32)
        rs2_in = dram.tile([NQB, EMBED, ROWS_T], f32)
        rs2_out = dram.tile([NQB, ROWS_T, ROWS_T], f32)
        x2_scr = dram.tile([ROWS_T, ROWS], f32)

        # ========== Stage 1+2 (fused phases): rmsnorm1 + qkv + attention ==========
        # Phase qb: rms+qkv for rows [512qb,512qb+512) then attention for q-block qb.
        # The next phase's square/mean chain is interleaved with this phase's
        # out-projection so the PE never starves on DVE latency; logits matmuls are
        # emitted one k-chunk ahead of the exp/mask chain.
        # rs1_in layout: row = (e//4)*2048 + qb*512 + (e%4)*128 + p, so each e-range
        # (er) forms a contiguous [2048, 512] block that ReduceScatters on its own as
        # soon as phase 3 finishes that e-range.
        with (
            tc.tile_pool(name="kv_store", bufs=1) as kv_pool,
            tc.tile_pool(name="s1", bufs=2) as s1,
            tc.tile_pool(name="s1ps", bufs=2, space="PSUM") as s1ps,
        ):
            k_store = kv_pool.tile([P, H_LOC, ROWS], f32r)
            v_store = kv_pool.tile([P, NR, H_LOC, HEAD_DIM], f32r)
            mask_sb = kv_pool.tile([P, QB + 3 * P], f32r)
            nc.sync.dma_start(mask_sb[:], masks[:].bitcast(f32r))

            xns = {}

            def emit_xn_dma(qb):
                xn = s1.tile([P, NE, QB], f32r, tag="xn", bufs=1, name=f"xn{qb}")
                nc.sync.dma_start(xn[:], r3(xt, slice(qb * QB, (qb + 1) * QB)).bitcast(f32r))
                xns[qb] = xn

            def emit_sq_ms_step(qb, e):
                """square + mean-accumulate for chunk e of phase qb."""
                if e == 0:
                    ms = s1ps.tile([1, QB], f32, tag="acc1", bufs=2, name=f"ms{qb}")
                    xns[(qb, "ms")] = ms
                ms = xns[(qb, "ms")]
                sq = s1.tile([P, QB], f32r, tag="sq", bufs=2)
                sl = xns[qb][:, e, :]
                nc.vector.tensor_mul(sq[:], sl, sl)
                nc.tensor.matmul(ms[:], ones_sb[:], sq[:],
                                 start=(e == 0), stop=(e == NE - 1))

            def emit_norm_tail(qb):
                ms = xns.pop((qb, "ms"))
                rsq = s1.tile([1, QB], f32, tag="rsq", bufs=1)
                nc.scalar.activation(rsq[:], ms[:], AF.Sqrt, bias=eps_sb[:], scale=1.0 / EMBED)
                rsq_i = s1.tile([1, QB], f32, tag="rsqi", bufs=1)
                nc.vector.reciprocal(rsq_i[:], rsq[:])
                bc = s1.tile([P, QB], f32r, tag="bc", bufs=2)
                nc.gpsimd.partition_broadcast(bc[:].bitcast(f32), rsq_i[:])
                xn = xns[qb]
                for e in range(NE):
                    sl = xn[:, e, :]
                    nc.vector.tensor_mul(sl, sl, bc[:])

            def emit_qkv(qb):
                xn = xns[qb]
                cols = slice(qb * QB, (qb + 1) * QB)
                q_ph = s1.tile([P, H_LOC, QB], f32r, tag="q_ph", bufs=1, name=f"q{qb}")
                for h in range(H_LOC):
                    wq_sb = s1.tile([P, NE, HEAD_DIM], f32r, tag="wqk", bufs=3)
                    nc.sync.dma_start(wq_sb[:], r3(wq[:, h, :]).bitcast(f32r))
                    wk_sb = s1.tile([P, NE, HEAD_DIM], f32r, tag="wqk", bufs=3)
                    nc.sync.dma_start(wk_sb[:], r3(wk[:, h, :]).bitcast(f32r))
                    q_ps = s1ps.tile([P, QB], f32, tag="proj", bufs=2)
                    for e in range(NE):
                        nc.tensor.matmul(q_ps[:], wq_sb[:, e, :], xn[:, e, :],
                                         start=(e == 0), stop=(e == NE - 1))
                    nc.vector.tensor_copy(q_ph[:, h, :], q_ps[:])
                    k_ps = s1ps.tile([P, QB], f32, tag="proj", bufs=2)
                    for e in range(NE):
                        nc.tensor.matmul(k_ps[:], wk_sb[:, e, :], xn[:, e, :],
                                         start=(e == 0), stop=(e == NE - 1))
                    nc.scalar.activation(k_store[:, h, cols], k_ps[:], AF.Copy)
                # v: e-outer with wv streamed; 4 row-chunk accumulators borrow
                # the lg/pv PSUM slots (idle between attention blocks)
                v_ps = [
                    s1ps.tile([P, H_LOC * HEAD_DIM], f32, tag=t, bufs=2,
                              name=f"v_ps{i}")
                    for i, t in enumerate(("lg", "lg", "pv", "pv"))
                ]
                for e in range(NE):
                    wv_e = s1.tile([P, H_LOC * HEAD_DIM], f32r, tag="wv_e", bufs=3)
                    nc.sync.dma_start(
                        wv_e[:], r3(wv)[:, e, :].bitcast(f32r))
                    for rc in range(QB // P):
                        nc.tensor.matmul(v_ps[rc][:], xn[:, e, rc * P : (rc + 1) * P],
                                         wv_e[:],
                                         start=(e == 0), stop=(e == NE - 1))
                for rc in range(QB // P):
                    rcg = qb * (QB // P) + rc
                    nc.vector.tensor_copy(
                        v_store[:, rcg].rearrange("p h d -> p (h d)"), v_ps[rc][:])
                return q_ph

            def emit_attention(qb, q_ph):
                ao_ph = s1.tile([P, H_LOC, QB], f32r, tag="ao_ph", bufs=1, name=f"ao{qb}")
                nk = (qb + 1) * (QB // P)
                for h in range(H_LOC):
                    pv_ps = s1ps.tile([P, QB], f32, tag="pv", bufs=2)
                    sum_ps = s1ps.tile([1, QB], f32, tag="acc1", bufs=2)
                    lg_tiles = {}

                    def emit_lg(kc):
                        lg = s1ps.tile([P, QB], f32, tag="lg", bufs=2)
                        nc.tensor.matmul(
                            lg[:], k_store[:, h, kc * P : (kc + 1) * P],
                            q_ph[:, h, :], start=True, stop=True)
                        lg_tiles[kc] = lg

                    emit_lg(0)
                    for kc in range(nk):
                        if kc + 1 < nk:
                            emit_lg(kc + 1)
                        lg = lg_tiles.pop(kc)
                        expt = s1.tile([P, QB], f32r, tag="expt", bufs=2)
                        nc.scalar.activation(expt[:], lg[:], AF.Exp, scale=INV_SQRT_D)
                        j = kc - qb * (QB // P)
                        if j >= 0:
                            off = (3 - j) * P
                            nc.vector.tensor_mul(expt[:], expt[:],
                                                 mask_sb[:, off : off + QB])
                        first, last = kc == 0, kc == nk - 1
                        nc.tensor.matmul(pv_ps[:], v_store[:, kc, h, :], expt[:],
                                         start=first, stop=last)
                        nc.tensor.matmul(sum_ps[:], ones_sb[:], expt[:],
                                         start=first, stop=last)
                    rec = s1.tile([1, QB], f32, tag="rec", bufs=2)
                    nc.vector.reciprocal(rec[:], sum_ps[:])
                    rbc = s1.tile([P, QB], f32r, tag="rbc", bufs=2)
                    nc.gpsimd.partition_broadcast(rbc[:].bitcast(f32), rec[:])
                    nc.vector.tensor_mul(ao_ph[:, h, :], pv_ps[:], rbc[:])
                return ao_ph

            def emit_outproj_step(qb, e, ao_ph):
                """one e-chunk of the out-projection partials of phase qb."""
                wo_sb = s1.tile([P, H_LOC, P], f32r, tag="wo", bufs=2)
                nc.sync.dma_start(
                    wo_sb[:], r3(wout, slice(e * P, (e + 1) * P)).bitcast(f32r))
                pr_ps = s1ps.tile([P, QB], f32, tag="proj", bufs=2)
                for c in range(H_LOC):
                    nc.tensor.matmul(pr_ps[:], wo_sb[:, c, :], ao_ph[:, c, :],
                                     start=(c == 0), stop=(c == H_LOC - 1))
                pr_sb = s1.tile([P, QB], f32, tag="pr_sb", bufs=2)
                nc.vector.tensor_copy(pr_sb[:], pr_ps[:])
                nc.sync.dma_start(
                    r3(rs1_in[qb][e * P : (e + 1) * P, :]), pr_sb[:])

            def emit_rs1(qb):
                nc.gpsimd.collective_compute(
                    "ReduceScatter", mybir.AluOpType.add, replica_groups=GROUPS,
                    ins=[rs1_in[qb][:].opt()], outs=[rs1_out[qb][:].opt()],
                )

            x2qs = {}

            def emit_stage3a(qb):
                """x2 = rs1_out + xte slice; mean-square partials -> tiny AllReduce."""
                cols = slice(qb * QB, (qb + 1) * QB)
                xe_sb = s1.tile([P, H_LOC, QB], f32, tag="xe", bufs=1, name=f"xe{qb}")
                nc.sync.dma_start(xe_sb[:], r3(xte, cols))
                x2q = s1.tile([P, H_LOC, QB], f32, tag="x2q", bufs=1, name=f"x2q{qb}")
                nc.sync.dma_start(x2q[:], r3(rs1_out[qb]))
                x2qs[qb] = x2q
                ms_part = s1ps.tile([1, QB], f32, tag="acc1", bufs=2, name=f"msp{qb}")
                for em in range(H_LOC):
                    nc.vector.tensor_add(x2q[:, em, :], x2q[:, em, :], xe_sb[:, em, :])
                    sq = s1.tile([P, QB], f32r, tag="sq", bufs=2)
                    nc.vector.tensor_mul(sq[:], x2q[:, em, :], x2q[:, em, :])
                    nc.tensor.matmul(ms_part[:], ones_sb[:], sq[:],
                                     start=(em == 0), stop=(em == H_LOC - 1))
                nc.sync.dma_start(r3(x2_scr, cols), x2q[:])
                ms_sb = s1.tile([1, QB], f32, tag="ms_sb", bufs=1)
                nc.vector.tensor_copy(ms_sb[:], ms_part[:])
                nc.sync.dma_start(ar_in[qb][:], ms_sb[:])
                nc.gpsimd.collective_compute(
                    "AllReduce", mybir.AluOpType.add, replica_groups=GROUPS,
                    ins=[ar_in[qb][:].opt()], outs=[ar_out[qb][:].opt()],
                )

            def emit_rsqn2(qb):
                """rsqrt of the AllReduced mean-square, normalize, ship to AG half."""
                cols_half = slice((qb % 2) * QB, (qb % 2 + 1) * QB)
                arv = s1.tile([1, QB], f32, tag="arv", bufs=1)
                nc.sync.dma_start(arv[:], ar_out[qb][:])
                rsq2 = s1.tile([1, QB], f32, tag="rsq2", bufs=1)
                nc.scalar.activation(rsq2[:], arv[:], AF.Sqrt, bias=eps_sb[:],
                                     scale=1.0 / EMBED)
                rsq2_i = s1.tile([1, QB], f32, tag="rsq2i", bufs=1)
                nc.vector.reciprocal(rsq2_i[:], rsq2[:])
                bc2 = s1.tile([P, QB], f32r, tag="bc", bufs=2)
                nc.gpsimd.partition_broadcast(bc2[:].bitcast(f32), rsq2_i[:])
                x2q = x2qs.pop(qb)
                ag_in = ag2a_in if qb < 2 else ag2b_in
                ag3 = r3(ag_in, cols_half)
                for em in range(H_LOC):
                    n2q = s1.tile([P, QB], f32r, tag="n2q", bufs=2)
                    nc.vector.tensor_mul(n2q[:], x2q[:, em, :], bc2[:])
                    nc.sync.dma_start(ag3[:, em, :].bitcast(f32r), n2q[:])

            def emit_ag2(half):
                i, o = (ag2a_in, ag2a_out) if half == 0 else (ag2b_in, ag2b_out)
                nc.gpsimd.collective_compute(
                    "AllGather", mybir.AluOpType.bypass, replica_groups=GROUPS,
                    ins=[i[:].opt()], outs=[o[:].opt()],
                )

            # ---- phase schedule (collectives pipelined under later phases) ----
            emit_xn_dma(0)
            for e in range(NE):
                emit_sq_ms_step(0, e)
            emit_norm_tail(0)
            aos = {}
            for qb in range(NQB):
                q_ph = emit_qkv(qb)
                aos[qb] = emit_attention(qb, q_ph)
                if qb >= 1:
                    emit_stage3a(qb - 1)
                if qb >= 2:
                    emit_rsqn2(qb - 2)
                if qb == 3:
                    emit_ag2(0)
                if qb + 1 < NQB:
                    emit_xn_dma(qb + 1)
                    for e in range(NE):
                        emit_sq_ms_step(qb + 1, e)
                        emit_outproj_step(qb, e, aos[qb])
                    emit_norm_tail(qb + 1)
                else:
                    for e in range(NE):
                        emit_outproj_step(qb, e, aos[qb])
                emit_rs1(qb)
            emit_stage3a(3)
            emit_rsqn2(2)
            emit_rsqn2(3)
            emit_ag2(1)

        # ========== Stage 5: FFN (hid shard), partials to per-group RS ==========
        with (
            tc.tile_pool(name="s5", bufs=1) as s5,
            tc.tile_pool(name="s5t", bufs=2) as s5t,
            tc.tile_pool(name="s5ps", bufs=2, space="PSUM") as s5ps,
        ):
            for rhh in range(ROWS // RH):  # 1024-row halves
                ag_out_h = ag2a_out if rhh == 0 else ag2b_out
                n2_sb = s5.tile([P, NE, RH], f32r, tag="n2h")
                nc.sync.dma_start(n2_sb[:], r3(ag_out_h).bitcast(f32r))
                act = s5.tile([P, NF, RH], f32r, tag="act")
                for f in range(NF):
                    wg_sb = s5t.tile([P, NE, P], f32r, tag="wgu", bufs=4)
                    nc.sync.dma_start(
                        wg_sb[:], r3(wg, slice(f * P, (f + 1) * P)).bitcast(f32r)
                    )
                    wu_sb = s5t.tile([P, NE, P], f32r, tag="wgu", bufs=4)
                    nc.sync.dma_start(
                        wu_sb[:], r3(wu, slice(f * P, (f + 1) * P)).bitcast(f32r)
                    )
                    for hb in range(2):
                        cols = slice(hb * QB, (hb + 1) * QB)
                        g_ps = s5ps.tile([P, QB], f32, tag="gate", bufs=2)
                        for e in range(NE):
                            nc.tensor.matmul(
                                g_ps[:], wg_sb[:, e, :], n2_sb[:, e, cols],
                                start=(e == 0), stop=(e == NE - 1),
                            )
                        u_ps = s5ps.tile([P, QB], f32, tag="up", bufs=2)
                        for e in range(NE):
                            nc.tensor.matmul(
                                u_ps[:], wu_sb[:, e, :], n2_sb[:, e, cols],
                                start=(e == 0), stop=(e == NE - 1),
                            )
                        gel = s5t.tile([P, QB], f32, tag="gel", bufs=3)
                        nc.scalar.activation(gel[:], g_ps[:], AF.Gelu_apprx_tanh)
                        nc.vector.tensor_mul(act[:, f, cols], gel[:], u_ps[:])
                for e in range(NE):
                    wd_sb = s5t.tile([P, NF, P], f32r, tag="wd", bufs=2)
                    nc.sync.dma_start(
                        wd_sb[:], r3(wd, slice(e * P, (e + 1) * P)).bitcast(f32r)
                    )
                    for hb in range(2):
                        g = rhh * 2 + hb
                        cols = slice(hb * QB, (hb + 1) * QB)
                        d_ps = s5ps.tile([P, QB], f32, tag="down", bufs=2)
                        for f in range(NF):
                            nc.tensor.matmul(
                                d_ps[:], wd_sb[:, f, :], act[:, f, cols],
                                start=(f == 0), stop=(f == NF - 1),
                            )
                        d_sb = s5t.tile([P, QB], f32, tag="dstage", bufs=3)
                        nc.vector.tensor_copy(d_sb[:], d_ps[:])
                        nc.sync.dma_start(
                            r3(rs2_in[g][e * P : (e + 1) * P, :]), d_sb[:])
                for hb in range(2):
                    g = rhh * 2 + hb
                    nc.gpsimd.collective_compute(
                        "ReduceScatter", mybir.AluOpType.add, replica_groups=GROUPS,
                        ins=[rs2_in[g][:].opt()], outs=[rs2_out[g][:].opt()],
                    )

        # ========== Stage 6: final residual per row-group (my E-slice) ==========
        with tc.tile_pool(name="s6", bufs=2) as s6:
            for g in range(NQB):
                cols = slice(g * QB, (g + 1) * QB)
                fsum = s6.tile([P, H_LOC, QB], f32, tag="fsum", bufs=2)
                nc.sync.dma_start(fsum[:], r3(rs2_out[g]))
                x2b = s6.tile([P, H_LOC, QB], f32, tag="x2b", bufs=2)
                nc.sync.dma_start(x2b[:], r3(x2_scr, cols))
                fin = s6.tile([P, H_LOC, QB], f32, tag="fin", bufs=2)
                nc.vector.tensor_add(fin[:], fsum[:], x2b[:])
                nc.sync.dma_start(r3(out_ext, cols), fin[:])


# ============================ host side ============================


def _prep_core_inputs(inputs):
    """Shard + transpose + fold rms scales into weights. Returns list of 8 in_maps."""
    x = np.asarray(inputs["x"], np.float32)          # [B, S, E]
    w_qkv = np.asarray(inputs["w_qkv"], np.float32)  # [E, H, 3D]
    w_out = np.asarray(inputs["w_out"], np.float32)  # [H, D, E]
    w_gate = np.asarray(inputs["w_gate"], np.float32)
    w_up = np.asarray(inputs["w_up"], np.float32)
    w_down = np.asarray(inputs["w_down"], np.float32)
    scale1 = np.asarray(inputs["scale1"], np.float32)
    scale2 = np.asarray(inputs["scale2"], np.float32)

    wqkv_s = w_qkv * scale1[:, None, None]
    wq_f = wqkv_s[:, :, 0:HEAD_DIM]
    wk_f = wqkv_s[:, :, HEAD_DIM : 2 * HEAD_DIM]
    wv_f = wqkv_s[:, :, 2 * HEAD_DIM : 3 * HEAD_DIM]
    wout_f = w_out.reshape(HEADS * HEAD_DIM, EMBED)
    wg_s = w_gate * scale2[:, None]
    wu_s = w_up * scale2[:, None]

    kp = np.arange(P)[:, None]
    m = np.arange(QB + 3 * P)[None, :]
    masks = (m >= kp + 3 * P).astype(np.float32)  # mask_j = masks[:, (3-j)*128 : (3-j)*128+512]

    in_maps = []
    for c in range(N_CORES):
        b, t = divmod(c, TP)
        hs = slice(H_LOC * t, H_LOC * (t + 1))
        fs = slice(F_LOC * t, F_LOC * (t + 1))
        xtb = np.ascontiguousarray(x[b].T)  # [E, S]
        in_maps.append(
            {
                "xt": xtb,
                "xte": np.ascontiguousarray(xtb[ROWS_T * t : ROWS_T * (t + 1), :]),
                "wq": np.ascontiguousarray(wq_f[:, hs, :]),
                "wk": np.ascontiguousarray(wk_f[:, hs, :]),
                "wv": np.ascontiguousarray(wv_f[:, hs, :].reshape(EMBED, H_LOC * HEAD_DIM)),
                "wout": np.ascontiguousarray(
                    wout_f[H_LOC * HEAD_DIM * t : H_LOC * HEAD_DIM * (t + 1), :]
                ),
                "wg": np.ascontiguousarray(wg_s[:, fs]),
                "wu": np.ascontiguousarray(wu_s[:, fs]),
                "wd": np.ascontiguousarray(w_down[fs, :]),
                "masks": np.ascontiguousarray(masks),
                "ones": np.ones((P, 1), np.float32),
            }
        )
    return in_maps


def _install_profile_hook():
    import sys
    import types

    try:
        import antenv.axon_hooks  # noqa: F401

        return
    except ImportError:
        pass
    try:
        from trn_agent_boot.trn_boot import _ntff_profile_via_ctypes

        _hook = _ntff_profile_via_ctypes("/opt/axon/libaxon_pjrt.so")
        _mod = types.ModuleType("antenv.axon_hooks")
        _mod.get_axon_ntff_profile_hook = lambda: _hook
        sys.modules["antenv.axon_hooks"] = _mod
    except Exception:
        pass


def _run(nc, in_maps, trace=False, trace_cores=None):
    _install_profile_hook()
    from concourse.bass_utils import run_bass_kernel_spmd

    return run_bass_kernel_spmd(
        nc,
        in_maps,
        core_ids=list(range(N_CORES)),
        trace=trace,
        trace_cores=trace_cores,
    )


def kernel(**inputs):
    if "nc" not in _NC_CACHE:
        _NC_CACHE["nc"] = build_kernel()
    nc = _NC_CACHE["nc"]
    in_maps = _prep_core_inputs(inputs)
    res = _run(nc, in_maps)
    out = np.empty((BATCH, SEQ, EMBED), np.float32)
    for c in range(N_CORES):
        b, t = divmod(c, TP)
        out[b, :, ROWS_T * t : ROWS_T * (t + 1)] = res.results[c]["out"].T
    return out


if __name__ == "__main__":
    build_kernel()
    print("build ok")
